# revision 1
# baseline (speedup 1.0000x reference)
"""Trainium2 Bass kernel for EvolveGCN-O forward (GCN message passing).

Math (reference):
    h   = x @ Wp + bp
    W   = LSTM-evolved weight from initial_weight (one step, h0=c0=IW)
    hw  = h @ W
    out = D^-1/2 (A+I) D^-1/2 hw + b_gcn

Factored for the kernel:
    out[d] = dinv[d] * (sum_{e: dst=d} dinv[src_e] * x[src_e]) @ (Wp @ W)
             + s2[d]*dinv[d]*(bp @ W) + b_gcn
with s2[d] = sum_{e in(d)} dinv[src_e] (self loops included as edges).

Distribution: nodes (dsts) sharded over 8 NeuronCores (serpentine by degree);
x table replicated; each core gathers its edges' source rows from a fp16
feature table in its HBM via SWDGE dma_gather (4 parallel queues), performs
the segment sum on the TensorEngine via one-hot masks built by the
VectorEngine, then applies the (tiny, replicated) evolved-weight matmul.
"""

import numpy as np

N_NODES = 10000
N_EDGES = 320000
IN_DIM = 128
HID = 256
M = 8                    # NeuronCores
NP = 10240               # padded node count (mult of 128)
RANKS = NP // 128        # 80
NPC = NP // M            # 1280 padded dsts per core
NGRP = NPC // 128        # 10 dst blocks of 128 per core
DUMMY_DL = 999.0         # dst_local value that matches no column

_cache = {}


def _round_up(x, m):
    return (x + m - 1) // m * m


def _build_module(ni_list):
    """Build+compile the Bacc module for given per-group edge counts."""
    import concourse.bacc as bacc
    import concourse.mybir as mybir
    import concourse.tile as tile

    TOT = int(sum(ni_list))
    nc = bacc.Bacc("TRN2", target_bir_lowering=False, debug=False,
                   num_devices=M, num_swdge_queues=4)
    f32, f16, i16 = mybir.dt.float32, mybir.dt.float16, mybir.dt.int16

    # ---- DRAM inputs ----
    x_t = nc.dram_tensor("x_tiled", [128, RANKS * 128], f32, kind="ExternalInput").ap()
    dinvT = nc.dram_tensor("dinvT", [128, RANKS], f32, kind="ExternalInput").ap()
    idx_in = nc.dram_tensor("idx", [128, TOT // 16], i16, kind="ExternalInput").ap()
    dl_in = nc.dram_tensor("dl", [128, TOT // 128], f16, kind="ExternalInput").ap()
    iota_in = nc.dram_tensor("iota", [128, 128], f16, kind="ExternalInput").ap()
    wiht_in = nc.dram_tensor("W_ihT", [256, 1024], f32, kind="ExternalInput").ap()
    whht_in = nc.dram_tensor("W_hhT", [256, 1024], f32, kind="ExternalInput").ap()
    iw_in = nc.dram_tensor("IW", [256, 256], f32, kind="ExternalInput").ap()
    iwt_in = nc.dram_tensor("IWT", [256, 256], f32, kind="ExternalInput").ap()
    wpt_in = nc.dram_tensor("WpT", [256, 128], f32, kind="ExternalInput").ap()
    bih_in = nc.dram_tensor("b_ih", [1, 1024], f32, kind="ExternalInput").ap()
    bhh_in = nc.dram_tensor("b_hh", [1, 1024], f32, kind="ExternalInput").ap()
    bp_in = nc.dram_tensor("bp_col", [256, 1], f32, kind="ExternalInput").ap()
    bgcn_in = nc.dram_tensor("b_gcn", [1, 256], f32, kind="ExternalInput").ap()
    ones_in = nc.dram_tensor("ones_row", [1, 128], f32, kind="ExternalInput").ap()
    s2_in = nc.dram_tensor("s2_row", [1, NPC], f32, kind="ExternalInput").ap()
    dri_in = nc.dram_tensor("dri_row", [1, NPC], f32, kind="ExternalInput").ap()
    dcol_in = nc.dram_tensor("dinv_col", [128, NGRP], f32, kind="ExternalInput").ap()

    out_t = nc.dram_tensor("out", [NPC, HID], f32, kind="ExternalOutput").ap()

    NXC = 4                      # xs build chunks
    RC = RANKS // NXC            # 20 ranks per chunk

    def gather_sbuf_nt(out_ap, in_ap, idxs_ap, num_idxs, queue_num):
        eng = nc.gpsimd
        in_ap = in_ap.bitcast(out_ap.dtype) if in_ap.dtype != out_ap.dtype else in_ap
        return eng.add_instruction(
            mybir.InstDMAGatherAnt(
                name=f"I-{nc.next_id()}",
                ins=[eng.lower_ap(in_ap), eng.lower_ap(idxs_ap),
                     eng.lower_val_access(eng.to_reg(num_idxs))],
                outs=[eng.lower_ap(out_ap)],
                transpose=False,
                num_idxs=num_idxs,
                elem_size=128,
                stride_bytes_256=0,
                gen_mode=0,
                single_packet=False,
                queue_num=queue_num,
                sbuf_tokens_per_rank=128,
                sbuf_free_dim_per_rank=256,
                sbuf_free_dim_pad_per_rank=0,
                sbuf_byte_offset=0,
            )
        )

    with tile.TileContext(nc) as tc:
        with (
            tc.tile_pool(name="stage", bufs=1) as stpool,
            tc.tile_pool(name="persist", bufs=1) as pp,
            tc.tile_pool(name="gp", bufs=5) as gpool,
            tc.tile_pool(name="sp", bufs=3) as spool,
            tc.tile_pool(name="psg", bufs=2, space="PSUM") as psg,
            tc.tile_pool(name="psl", bufs=1, space="PSUM") as psl,
        ):
            # ---------- phase 0: x + dinv + idx loads first, xs build ASAP ----
            dinv_sb = pp.tile([128, RANKS], f32)
            nc.sync.dma_start(out=dinv_sb[:], in_=dinvT[:])
            xs_sb = pp.tile([128, RANKS * 128], f16)
            idxs = pp.tile([128, TOT // 16], i16)
            dls = pp.tile([128, TOT // 128], f16)
            iota = pp.tile([128, 128], f16)
            xsts = []
            for cidx in range(NXC):
                xst = stpool.tile([128, RC * 128], f32, tag=f"xstage{cidx % 2}")
                eng = nc.sync if cidx % 2 == 0 else nc.scalar
                eng.dma_start(
                    out=xst[:], in_=x_t[:, cidx * RC * 128:(cidx + 1) * RC * 128])
                xsts.append(xst)
            nc.scalar.dma_start(out=idxs[:], in_=idx_in[:])
            xs_tts = []
            for cidx in range(NXC):
                xs_tts.append(nc.vector.tensor_tensor(
                    out=xs_sb[:, cidx * RC * 128:(cidx + 1) * RC * 128]
                        .rearrange("p (r f) -> p r f", f=128),
                    in0=xsts[cidx][:].rearrange("p (r f) -> p r f", f=128),
                    in1=dinv_sb[:, cidx * RC:(cidx + 1) * RC]
                        .rearrange("p (r o) -> p r o", o=1)
                        .to_broadcast([128, RC, 128]),
                    op=mybir.AluOpType.mult,
                ))
            # dl/iota arrive after the table build so the S_T is_equal ops
            # cannot preempt the critical xs TT chain on Vector
            from concourse.tile import add_dep_helper
            dl_ld = nc.sync.dma_start(out=dls[:], in_=dl_in[:])
            io_ld = nc.sync.dma_start(out=iota[:], in_=iota_in[:])
            add_dep_helper(dl_ld.ins, xs_tts[-1].ins, reason="defer S_T builds")
            add_dep_helper(io_ld.ins, xs_tts[-1].ins, reason="defer S_T builds")

            def emit_weights():
                # ---------- small-tensor loads ----------
                wiht = pp.tile([128, 2, 1024], f32)
                whht = pp.tile([128, 2, 1024], f32)
                iw = pp.tile([128, 2, 256], f32)
                iwt = pp.tile([128, 2, 256], f32)
                wpt = pp.tile([128, 2, 128], f32)
                bih = pp.tile([1, 1024], f32)
                bhh = pp.tile([1, 1024], f32)
                bp_c = pp.tile([128, 2, 1], f32)
                bgcn = pp.tile([1, 256], f32)
                ones = pp.tile([1, 128], f32)
                s2r = pp.tile([1, NPC], f32)
                drir = pp.tile([1, NPC], f32)
                dcol = pp.tile([128, NGRP], f32)
                for t_, s_ in ((wiht, wiht_in), (whht, whht_in), (iw, iw_in),
                               (iwt, iwt_in), (wpt, wpt_in), (bp_c, bp_in)):
                    nc.sync.dma_start(out=t_[:], in_=s_.rearrange("(k p) c -> p k c", p=128))
                for t_, s_ in ((bih, bih_in), (bhh, bhh_in), (bgcn, bgcn_in),
                               (ones, ones_in), (s2r, s2_in), (drir, dri_in),
                               (dcol, dcol_in)):
                    nc.sync.dma_start(out=t_[:], in_=s_[:])

                # ---------- LSTM weight evolution (tiny, replicated) ----------
                wsum = pp.tile([128, 2, 1024], f32)
                nc.vector.tensor_tensor(out=wsum[:], in0=wiht[:], in1=whht[:],
                                        op=mybir.AluOpType.add)
                bsum = pp.tile([1, 1024], f32)
                nc.vector.tensor_tensor(out=bsum[:], in0=bih[:], in1=bhh[:],
                                        op=mybir.AluOpType.add)
                w_ev = pp.tile([128, 2, 256], f32)   # evolved GCN weight W
                for ic in range(2):
                    gpsum = psl.tile([128, 1024], f32, space="PSUM", tag="gates")
                    for h in range(2):
                        gs = slice(512 * h, 512 * (h + 1))
                        nc.tensor.matmul(out=gpsum[:, gs], lhsT=ones[:, :],
                                         rhs=bsum[:, gs], start=True, stop=False)
                        nc.tensor.matmul(out=gpsum[:, gs],
                                         lhsT=iwt[:, 0, 128 * ic:128 * (ic + 1)],
                                         rhs=wsum[:, 0, gs], start=False, stop=False)
                        nc.tensor.matmul(out=gpsum[:, gs],
                                         lhsT=iwt[:, 1, 128 * ic:128 * (ic + 1)],
                                         rhs=wsum[:, 1, gs], start=False, stop=True)
                    Sig = mybir.ActivationFunctionType.Sigmoid
                    Tanh = mybir.ActivationFunctionType.Tanh
                    si = stpool.tile([128, 256], f32, tag="si")
                    sf = stpool.tile([128, 256], f32, tag="sf")
                    tg = stpool.tile([128, 256], f32, tag="tg")
                    so = stpool.tile([128, 256], f32, tag="so")
                    nc.scalar.activation(out=si[:], in_=gpsum[:, 0:256], func=Sig)
                    nc.scalar.activation(out=sf[:], in_=gpsum[:, 256:512], func=Sig)
                    nc.scalar.activation(out=tg[:], in_=gpsum[:, 512:768], func=Tanh)
                    nc.scalar.activation(out=so[:], in_=gpsum[:, 768:1024], func=Sig)
                    c1 = stpool.tile([128, 256], f32, tag="c1")
                    nc.vector.tensor_tensor(out=c1[:], in0=sf[:],
                                            in1=iw[:, ic, :],
                                            op=mybir.AluOpType.mult)
                    c2 = stpool.tile([128, 256], f32, tag="c2")
                    nc.vector.tensor_tensor(out=c2[:], in0=si[:], in1=tg[:],
                                            op=mybir.AluOpType.mult)
                    cc = stpool.tile([128, 256], f32, tag="cc")
                    nc.vector.tensor_tensor(out=cc[:], in0=c1[:], in1=c2[:],
                                            op=mybir.AluOpType.add)
                    tcc = stpool.tile([128, 256], f32, tag="tcc")
                    nc.scalar.activation(out=tcc[:], in_=cc[:], func=Tanh)
                    nc.vector.tensor_tensor(out=w_ev[:, ic, :],
                                            in0=so[:], in1=tcc[:],
                                            op=mybir.AluOpType.mult)

                wpw = pp.tile([128, 256], f32)       # Wp @ W
                wp_ps = psl.tile([128, 256], f32, space="PSUM", tag="wpw")
                nc.tensor.matmul(out=wp_ps[:], lhsT=wpt[:, 0, :], rhs=w_ev[:, 0, :],
                                 start=True, stop=False)
                nc.tensor.matmul(out=wp_ps[:], lhsT=wpt[:, 1, :], rhs=w_ev[:, 1, :],
                                 start=False, stop=True)
                nc.vector.tensor_copy(out=wpw[:], in_=wp_ps[:])
                bpw = pp.tile([1, 256], f32)         # bp @ W
                bp_ps = psl.tile([1, 256], f32, space="PSUM", tag="bpw")
                nc.tensor.matmul(out=bp_ps[:], lhsT=bp_c[:, 0, :], rhs=w_ev[:, 0, :],
                                 start=True, stop=False)
                nc.tensor.matmul(out=bp_ps[:], lhsT=bp_c[:, 1, :], rhs=w_ev[:, 1, :],
                                 start=False, stop=True)
                nc.vector.tensor_copy(out=bpw[:], in_=bp_ps[:])

                return s2r, drir, dcol, wpw, bpw, bgcn

            # ---------- main: gather + one-hot + PE segment sum ----------
            xaggT = pp.tile([128, NPC], f32)
            ioff = 0
            coff = 0
            qrr = 0
            for g in range(NGRP):
                ni = int(ni_list[g])
                nch = ni // 128
                gbuf = gpool.tile([128, nch, 128], f16, tag="gbuf")
                # split into <=4096-idx gather calls (SWDGE ring capacity)
                done = 0
                while done < ni:
                    part = min(1536, ni - done)
                    gather_sbuf_nt(
                        gbuf[:, done // 128:(done + part) // 128, :],
                        xs_sb[:],
                        idxs[:, ioff + done // 16:ioff + (done + part) // 16],
                        part,
                        qrr % 4,
                    )
                    qrr += 1
                    done += part
                if g == 0:
                    s2r, drir, dcol, wpw, bpw, bgcn = emit_weights()
                st = spool.tile([128, nch, 128], f16, tag="st")
                nc.vector.tensor_tensor(
                    out=st[:],
                    in0=dls[:, coff:coff + nch]
                        .rearrange("p (c o) -> p c o", o=1)
                        .to_broadcast([128, nch, 128]),
                    in1=iota[:].rearrange("p (o d) -> p o d", o=1)
                        .to_broadcast([128, nch, 128]),
                    op=mybir.AluOpType.is_equal,
                )
                gps = psg.tile([128, 128], f32, space="PSUM", tag="agg")
                for c in range(nch):
                    nc.tensor.matmul(out=gps[:], lhsT=gbuf[:, c, :],
                                     rhs=st[:, c, :],
                                     start=(c == 0), stop=(c == nch - 1))
                nc.scalar.activation(out=xaggT[:, g * 128:(g + 1) * 128],
                                     in_=gps[:],
                                     func=mybir.ActivationFunctionType.Copy)
                ioff += ni // 16
                coff += nch

                # final: out rows = dinv*(xagg@WpW + s2*bpW + dri*bgcn)
                ops = psg.tile([128, HID], f32, space="PSUM", tag="ops")
                ds = slice(128 * g, 128 * (g + 1))
                nc.tensor.matmul(out=ops[:], lhsT=s2r[:, ds], rhs=bpw[:],
                                 start=True, stop=False)
                nc.tensor.matmul(out=ops[:], lhsT=drir[:, ds], rhs=bgcn[:],
                                 start=False, stop=False)
                nc.tensor.matmul(out=ops[:], lhsT=xaggT[:, ds], rhs=wpw[:],
                                 start=False, stop=True)
                orow = spool.tile([128, HID], f32, tag="orow")
                nc.scalar.activation(out=orow[:], in_=ops[:],
                                     func=mybir.ActivationFunctionType.Copy,
                                     scale=dcol[:, g:g + 1])
                nc.sync.dma_start(
                    out=out_t.rearrange("(g p) h -> g p h", p=128)[g],
                    in_=orow[:],
                )

    nc.compile()
    return nc


def _preprocess(edge_index):
    """Host-side index preprocessing. Returns per-core index structures."""
    src = np.asarray(edge_index[0], dtype=np.int64)
    dst = np.asarray(edge_index[1], dtype=np.int64)
    loops = np.arange(N_NODES, dtype=np.int64)
    src_all = np.concatenate([src, loops])
    dst_all = np.concatenate([dst, loops])
    deg = np.bincount(dst_all, minlength=N_NODES).astype(np.float64)
    dinv = (1.0 / np.sqrt(deg)).astype(np.float32)

    # serpentine assignment of degree-sorted nodes to cores
    order = np.argsort(-deg, kind="stable")
    r = np.arange(N_NODES)
    rr = r % (2 * M)
    core_r = np.where(rr < M, rr, 2 * M - 1 - rr)
    lrank_r = (r // (2 * M)) * 2 + (rr >= M)
    core_of = np.empty(N_NODES, np.int64)
    lrank_of = np.empty(N_NODES, np.int64)
    core_of[order] = core_r
    lrank_of[order] = lrank_r

    # per-core permutation: perm[c][l] = global node at local rank l
    perm = np.empty((M, N_NODES // M), np.int64)
    perm[core_of[order], lrank_of[order]] = order

    # edges keyed by (core, block, dst_local)
    e_core = core_of[dst_all]
    e_lrank = lrank_of[dst_all]
    e_block = e_lrank // 128
    e_dl = e_lrank % 128
    key = (e_core * NGRP + e_block) * 128 + e_dl
    eorder = np.argsort(key, kind="stable")
    cnt = np.bincount(e_core * NGRP + e_block, minlength=M * NGRP).reshape(M, NGRP)
    ni_list = np.maximum(_round_up(cnt.max(axis=0), 128), 128).astype(np.int64)
    TOT = int(ni_list.sum())

    # SBUF table tokens are node ids (partition n%128, rank n//128)
    tok_of = np.arange(NP).astype(np.int16)
    dummy_tok = tok_of[N_NODES]  # a zero row

    tok_arr = np.full((M, TOT), dummy_tok, np.int16)
    dl_arr = np.full((M, TOT), DUMMY_DL, np.float16)
    goff = np.concatenate([[0], np.cumsum(ni_list)])
    s_tok = tok_of[src_all[eorder]]
    s_dl = e_dl[eorder].astype(np.float16)
    s_core = e_core[eorder]
    s_block = e_block[eorder]
    bounds = np.searchsorted(s_core * NGRP + s_block, np.arange(M * NGRP + 1), side="left")
    for c in range(M):
        for g in range(NGRP):
            b0, b1 = bounds[c * NGRP + g], bounds[c * NGRP + g + 1]
            n = b1 - b0
            # sort the group's edges by token so the gather's HBM reads
            # sweep nearly linearly (dst identity is carried by dl)
            o = np.argsort(s_tok[b0:b1], kind="stable")
            tok_arr[c, goff[g]:goff[g] + n] = s_tok[b0:b1][o]
            dl_arr[c, goff[g]:goff[g] + n] = s_dl[b0:b1][o]

    # s2[d] = sum over in-edges of dinv[src] (self loop included)
    s2 = np.bincount(dst_all, weights=dinv[src_all].astype(np.float64),
                     minlength=N_NODES).astype(np.float32)

    # wrap indices: per group, idx i -> [i%16, goff/16 + i//16]; replicate x8
    idx_w = np.zeros((M, 16, TOT // 16), np.int16)
    dl_t = np.zeros((M, 128, TOT // 128), np.float16)
    for g in range(NGRP):
        ni = int(ni_list[g])
        i = np.arange(ni)
        seg = tok_arr[:, goff[g]:goff[g] + ni]
        idx_w[:, i % 16, goff[g] // 16 + i // 16] = seg
        dseg = dl_arr[:, goff[g]:goff[g] + ni]
        dl_t[:, i % 128, goff[g] // 128 + i // 128] = dseg
    idx_rep = np.tile(idx_w, (1, 8, 1))

    return dict(dinv=dinv, perm=perm, ni_list=ni_list, idx_rep=idx_rep,
                dl_t=dl_t, s2=s2)


LAST_RESULT = None


def kernel(x, edge_index, Wp, bp, W_ih, W_hh, b_ih, b_hh, initial_weight, b_gcn):
    global LAST_RESULT
    from concourse.bass_utils import run_bass_kernel_spmd

    x = np.asarray(x, np.float32)
    Wp = np.asarray(Wp, np.float32)
    bp = np.asarray(bp, np.float32)
    W_ih = np.asarray(W_ih, np.float32)
    W_hh = np.asarray(W_hh, np.float32)
    b_ih = np.asarray(b_ih, np.float32)
    b_hh = np.asarray(b_hh, np.float32)
    initial_weight = np.asarray(initial_weight, np.float32)
    b_gcn = np.asarray(b_gcn, np.float32)
    assert x.shape == (N_NODES, IN_DIM)

    pre = _preprocess(edge_index)
    dinv, perm, ni_list, s2 = pre["dinv"], pre["perm"], pre["ni_list"], pre["s2"]

    key = tuple(int(v) for v in ni_list)
    if key not in _cache:
        _cache[key] = _build_module(key)
    nc = _cache[key]

    # shared tensors
    xp = np.zeros((NP, IN_DIM), np.float32)
    xp[:N_NODES] = x
    x_tiled = np.ascontiguousarray(
        xp.reshape(RANKS, 128, IN_DIM).transpose(1, 0, 2).reshape(128, RANKS * 128))
    dinvp = np.zeros(NP, np.float32)
    dinvp[:N_NODES] = dinv
    dinvT = np.ascontiguousarray(dinvp.reshape(RANKS, 128).T)
    iota_np = np.tile(np.arange(128, dtype=np.float16)[None, :], (128, 1))
    shared = {
        "x_tiled": x_tiled,
        "dinvT": dinvT,
        "iota": np.ascontiguousarray(iota_np),
        "W_ihT": np.ascontiguousarray(W_ih.T),
        "W_hhT": np.ascontiguousarray(W_hh.T),
        "IW": initial_weight,
        "IWT": np.ascontiguousarray(initial_weight.T),
        "WpT": np.ascontiguousarray(Wp.T),
        "b_ih": b_ih.reshape(1, -1),
        "b_hh": b_hh.reshape(1, -1),
        "bp_col": np.ascontiguousarray(bp.reshape(-1, 1)),
        "b_gcn": b_gcn.reshape(1, -1),
        "ones_row": np.ones((1, 128), np.float32),
    }
    NLOC = N_NODES // M
    in_maps = []
    for c in range(M):
        pc = perm[c]
        s2p = np.zeros(NPC, np.float32)
        s2p[:NLOC] = s2[pc]
        drip = np.zeros(NPC, np.float32)
        drip[:NLOC] = 1.0 / dinv[pc]
        dlocp = np.zeros(NPC, np.float32)
        dlocp[:NLOC] = dinv[pc]
        in_maps.append({
            **shared,
            "idx": np.ascontiguousarray(pre["idx_rep"][c]),
            "dl": np.ascontiguousarray(pre["dl_t"][c]),
            "s2_row": s2p.reshape(1, -1),
            "dri_row": drip.reshape(1, -1),
            "dinv_col": np.ascontiguousarray(dlocp.reshape(NGRP, 128).T),
        })

    res = run_bass_kernel_spmd(nc, in_maps, list(range(M)))
    LAST_RESULT = res

    out = np.empty((N_NODES, HID), np.float32)
    for c in range(M):
        out[perm[c]] = res.results[c]["out"][:NLOC]
    return out

